# revision 1
# baseline (speedup 1.0000x reference)
"""GCN autoencoder (6x gcn_layer) on 8 TRN2 NeuronCores.

Strategy:
  - Rows of adj_/X sharded across 8 cores; weights replicated.
  - All device tensors bf16 (fp32 PSUM accumulation); host does the free
    sharding / transposes / casts and the final gather+transpose.
  - adj-mm produces zT = (adj_shard @ H)^T so the next layer's XW matmul
    consumes it directly (no transposes anywhere on device).
  - Each layer computes two row-phases (512 local rows each).  After a
    phase: XW(l+1) for those rows -> DRAM bounce -> AllGather -> next
    layer's H chunks; the consumer accumulates its 64 k-chunks in
    arrival-wave order so the second gather's flight hides under the
    first wave's matmuls.
  - Gathered-H / adj-resident / H1 buffers are split per-wave / quartered
    so a reader only depends on the writes that produced its chunk.
  - adj columns 0:512 SBUF-resident (the full bf16 shard does not fit
    beside the H buffers); 512:1024 streamed per layer in k-chunk pairs.
  - Layer 1's H1 = X @ W1 is computed fully on every core from the
    (replicated, free) input X -> no collective before the first adj-mm.
  - Two small warmup AllGathers absorb the collective stream's first-use
    cost while the CC queue is otherwise idle.
  (The ZG z-gather path is disabled: building H locally from a gathered
   z deepened the post-landing critical chain and measured slower.)
"""

import sys

import numpy as np

if "/opt/trn_rl_repo" not in sys.path:
    sys.path.insert(0, "/opt/trn_rl_repo")

import ml_dtypes

import concourse.bacc as bacc
import concourse.tile as tile
from concourse import mybir
from concourse.bass_utils import run_bass_kernel_spmd

N = 8192
D_IN = 512
NCORES = 8
R = N // NCORES  # 1024 rows per core
DIMS = [(512, 256), (256, 256), (256, 128), (128, 256), (256, 256), (256, 512)]

BF16 = mybir.dt.bfloat16
F32 = mybir.dt.float32
NP_BF16 = ml_dtypes.bfloat16
RELU = mybir.ActivationFunctionType.Relu

KO = N // 128  # 64 k-chunks over the gather dim
RT = R // 128  # 8 local row tiles
NPH = 2
PH = R // NPH  # 512 rows per phase
HALF = RT // NPH  # 4 chunks each core contributes per phase
ZG = -1  # disabled: z-gather deepened the post-landing critical chain  # layer index (0-based) whose H is built locally from gathered z

_CACHED = {}


def _build():
    nc = bacc.Bacc(
        "TRN2",
        target_bir_lowering=False,
        debug=False,
        enable_asserts=False,
        num_devices=NCORES,
    )

    adjT = nc.dram_tensor("adjT", [N, R], BF16, kind="ExternalInput")
    xT = nc.dram_tensor("xT", [D_IN, N], BF16, kind="ExternalInput")
    w_dram = [
        nc.dram_tensor(f"W{i + 1}", list(DIMS[i]), BF16, kind="ExternalInput")
        for i in range(6)
    ]
    outT = nc.dram_tensor("outT", [DIMS[-1][1], R], F32, kind="ExternalOutput")

    adjT_r = adjT.ap().rearrange("(ko p) r -> p ko r", p=128)
    xT_r = xT.ap().rearrange("(kx p) c -> p kx c", p=128)

    with tile.TileContext(nc) as tc:
        with (
            tc.tile_pool(name="adjres", bufs=1) as adjres_p,
            tc.tile_pool(name="adjstr", bufs=5) as adjstr_p,
            tc.tile_pool(name="wp", bufs=1) as wp,
            tc.tile_pool(name="xtp", bufs=3) as xtp,
            tc.tile_pool(name="ztgp", bufs=3) as ztgp,
            tc.tile_pool(name="ztp", bufs=8) as ztp,
            tc.tile_pool(name="hp", bufs=6) as hp,
            tc.tile_pool(name="hstage", bufs=4) as hstage,
            tc.tile_pool(name="ostage", bufs=2) as ostage,
            tc.tile_pool(name="psz", bufs=6, space="PSUM") as psz,
            tc.tile_pool(name="psh", bufs=2, space="PSUM") as psh,
            tc.tile_pool(name="dram", bufs=1, space="DRAM") as dram,
        ):
            # ---- resident weights ----
            w_sb = []
            for i, (di, do) in enumerate(DIMS):
                w_t = wp.tile([128, di // 128, do], BF16, name=f"w{i}_sb")
                nc.sync.dma_start(
                    w_t[:], w_dram[i].ap().rearrange("(kx p) n -> p kx n", p=128)
                )
                w_sb.append(w_t)

            # warmup AllGathers: absorb the collective-stream first-use cost
            # while the CC queue is otherwise idle (overlaps XW1 / barrier)
            for wi, wrows in enumerate((16, PH)):
                wu_in = dram.tile([wrows, 256], BF16, tag=f"wu{wi}i",
                                  name=f"wu{wi}i")
                wu_out = dram.tile([NCORES * wrows, 256], BF16,
                                   addr_space="Shared", tag=f"wu{wi}o",
                                   name=f"wu{wi}o")
                nc.gpsimd.collective_compute(
                    "AllGather",
                    mybir.AluOpType.bypass,
                    ins=[wu_in[:].opt()],
                    outs=[wu_out[:].opt()],
                    replica_groups=[list(range(NCORES))],
                )

            # resident adj columns 0:512, quartered so early k-chunk reads
            # only wait on their quarter's DMA; 512:1024 streamed per layer
            adj_res = [
                adjres_p.tile([128, 16, PH], BF16, name=f"adj_res{q}")
                for q in range(4)
            ]
            adj_stream_cache = {}

            def adj_mov(g, n):
                if n == 0:
                    return adj_res[g // 16][:, g % 16, :]
                # pairs: every consumption segment (waves and the
                # half-wave insert) covers complete g//2 pairs, so a pair's
                # pool slot is never revisited after its segment
                grp = g // 2
                t = adj_stream_cache.get(grp)
                if t is None:
                    t = adjstr_p.tile([128, 2, PH], BF16, tag="adjs",
                                      name=f"as{grp}")
                    nc.sync.dma_start(
                        t[:], adjT_r[:, grp * 2 : grp * 2 + 2, PH:R]
                    )
                    adj_stream_cache[grp] = t
                return t[:, g % 2, :]

            # ---- layer 1: H1 = X @ W1 computed fully on every core ----
            # quartered: [128, 16, 256] x4; read of chunk g -> quarter g//16
            h1 = [
                hp.tile([128, 16, DIMS[0][1]], BF16, tag="h", name=f"h1_{q}")
                for q in range(4)
            ]
            for g0 in range(0, KO, 2):
                xt_t = xtp.tile([128, D_IN // 128, 256], BF16, tag="xt")
                nc.sync.dma_start(xt_t[:], xT_r[:, :, g0 * 128 : g0 * 128 + 256])
                for g in (g0, g0 + 1):
                    ps_h = psh.tile([128, DIMS[0][1]], F32, tag="psh")
                    for kx in range(D_IN // 128):
                        c = (g - g0) * 128
                        nc.tensor.matmul(
                            ps_h[:],
                            xt_t[:, kx, c : c + 128],
                            w_sb[0][:, kx, :],
                            start=(kx == 0),
                            stop=(kx == D_IN // 128 - 1),
                        )
                    nc.vector.tensor_copy(h1[g // 16][:, g % 16, :], ps_h[:])

            def h1_read(m, g):
                return h1[g // 16][:, g % 16, m * 128 : (m + 1) * 128]

            h_read = h1_read

            # resident-adj load, emitted after the XW1 stream so the small
            # xT/W DMAs get the queues first; k-ordered to match consumption
            for q in range(4):
                for j in range(0, 16, 4):
                    nc.sync.dma_start(
                        adj_res[q][:, j : j + 4, :],
                        adjT_r[:, q * 16 + j : q * 16 + j + 4, 0:PH],
                    )

            # consumption waves: layer 1 in production order (g ascending);
            # layers >=2 by producer phase ({c*8 + n*4 + j, j<4} per phase n)
            waves_l1 = [list(range(KO // 2)), list(range(KO // 2, KO))]
            waves_g = [
                [c * RT + n * HALF + j
                 for c in range(NCORES) for j in range(HALF)]
                for n in range(NPH)
            ]

            z_gaths = {}  # producer phase n -> gathered zT DRAM buffer

            for li, (di, do) in enumerate(DIMS):
                last = li == len(DIMS) - 1
                mt = do // 128
                kwaves = waves_l1 if li == 0 else waves_g
                adj_stream_cache.clear()
                gather_z = (li + 1 == ZG)  # this layer's output z is gathered

                if not last:
                    di2, do2 = DIMS[li + 1]
                    kxn2 = di2 // 128  # == mt
                    # per-wave (and per-column-half for do2=512) H buffers:
                    # h_next[ci][w] holds chunks {c*8 + w*4 + j} at pos c*4+j
                    ncs = 1 if do2 <= 256 else 2
                    dc2 = do2 if do2 <= 256 else 256
                    h_next = [
                        [hp.tile([128, KO // 2, dc2], BF16, tag="h",
                                 name=f"h{li + 2}_{ci}_{w}")
                         for w in range(NPH)]
                        for ci in range(ncs)
                    ]

                    def make_reader(h_tiles, split):
                        def rd(m, g):
                            ci, mc = (m // 2, m % 2) if split else (0, m)
                            c, r8 = g // 8, g % 8
                            w, j = r8 // 4, r8 % 4
                            return h_tiles[ci][w][:, c * 4 + j,
                                                  mc * 128 : (mc + 1) * 128]
                        return rd

                def build_h_wave(w):
                    # this layer's H chunks for wave w = gathered_z @ W,
                    # computed locally as the wave's gather lands
                    gz_r = z_gaths[w].rearrange(
                        "(c kx p) r -> c p kx r", c=NCORES, p=128
                    )
                    kxn = di // 128
                    for c in range(NCORES):
                        ztg = ztgp.tile([128, kxn, PH], BF16, tag="ztg",
                                        name=f"ztg{w}_{c}")
                        nc.gpsimd.dma_start(ztg[:], gz_r[c])
                        for j in range(HALF):
                            ps_hx = psh.tile([128, do], F32, tag="psh")
                            for kx in range(kxn):
                                nc.tensor.matmul(
                                    ps_hx[:],
                                    ztg[:, kx, j * 128 : (j + 1) * 128],
                                    w_sb[li][:, kx, :],
                                    start=(kx == 0),
                                    stop=(kx == kxn - 1),
                                )
                            for ci in range(len(h_tiles_cur)):
                                c0 = ci * 256
                                dc = min(256, do - c0)
                                nc.vector.tensor_copy(
                                    h_tiles_cur[ci][w][:, c * 4 + j, :],
                                    ps_hx[:, c0 : c0 + dc],
                                )

                ps_zs = [[psz.tile([128, PH], F32, tag="psz", name=f"psz{n}_{m}")
                          for m in range(mt)] for n in range(NPH)]
                mm_cnt = [[0] * mt for _ in range(NPH)]

                def emit_block(wb, n, lo=0, hi=None):
                    for g in kwaves[wb][lo:hi]:
                        mov = adj_mov(g, n)
                        for m in range(mt):
                            nc.tensor.matmul(
                                ps_zs[n][m][:],
                                h_read(m, g),
                                mov,
                                start=(mm_cnt[n][m] == 0),
                                stop=(mm_cnt[n][m] == KO - 1),
                            )
                            mm_cnt[n][m] += 1

                def emit_epilogue(n):
                    zt_p = []
                    for m in range(mt):
                        if last:
                            o_st = ostage.tile([128, PH], F32, tag="ost")
                            nc.scalar.activation(o_st[:], ps_zs[n][m][:], RELU)
                            nc.sync.dma_start(
                                outT[m * 128 : (m + 1) * 128,
                                     n * PH : (n + 1) * PH],
                                o_st[:],
                            )
                        elif gather_z:
                            z_st = hstage.tile([128, PH], BF16, tag="hst")
                            nc.scalar.activation(z_st[:], ps_zs[n][m][:], RELU)
                            nc.scalar.dma_start(
                                zbounce[m * 128 : (m + 1) * 128, :], z_st[:]
                            )
                        else:
                            z_t = ztp.tile([128, PH], BF16, tag="zt",
                                           name=f"z{li + 1}_{m}_{n}")
                            nc.scalar.activation(z_t[:], ps_zs[n][m][:], RELU)
                            zt_p.append(z_t)
                    if last:
                        return
                    if gather_z:
                        # gather zT itself; the consumer builds H locally
                        gz = dram.tile(
                            [NCORES * do, PH], BF16, addr_space="Shared",
                            tag=f"zg{n}", name=f"zg{n}",
                        )
                        nc.gpsimd.collective_compute(
                            "AllGather",
                            mybir.AluOpType.bypass,
                            ins=[zbounce[:].opt()],
                            outs=[gz[:].opt()],
                            replica_groups=[list(range(NCORES))],
                        )
                        z_gaths[n] = gz
                        return
                    # XW(l+1) for this phase's rows -> bounce -> AllGather
                    bounce = dram.tile([PH, do2], BF16, tag=f"hb{li}_{n}",
                                       name=f"hb{li}_{n}")
                    for j in range(HALF):
                        ps_h = psh.tile([128, do2], F32, tag="psh")
                        for kx in range(kxn2):
                            nc.tensor.matmul(
                                ps_h[:],
                                zt_p[kx][:, j * 128 : (j + 1) * 128],
                                w_sb[li + 1][:, kx, :],
                                start=(kx == 0),
                                stop=(kx == kxn2 - 1),
                            )
                        h_st = hstage.tile([128, do2], BF16, tag="hst")
                        nc.vector.tensor_copy(h_st[:], ps_h[:])
                        nc.sync.dma_start(
                            bounce[j * 128 : (j + 1) * 128, :], h_st[:]
                        )
                    gath = dram.tile(
                        [NCORES * PH, do2], BF16, addr_space="Shared",
                        tag=f"hg{li}_{n}", name=f"hg{li}_{n}",
                    )
                    nc.gpsimd.collective_compute(
                        "AllGather",
                        mybir.AluOpType.bypass,
                        ins=[bounce[:].opt()],
                        outs=[gath[:].opt()],
                        replica_groups=[list(range(NCORES))],
                    )
                    g_r = gath.rearrange("(q p) d -> p q d", p=128)
                    for ci in range(len(h_next)):
                        c0 = ci * 256
                        dc = min(256, do2 - c0)
                        for c in range(NCORES):
                            nc.sync.dma_start(
                                h_next[ci][n][:, c * HALF : (c + 1) * HALF, :],
                                g_r[:, c * HALF : (c + 1) * HALF, c0 : c0 + dc],
                            )

                if gather_z:
                    zbounce = dram.tile([do, PH], BF16, tag="zb0",
                                        name=f"zb{li}_0")
                if li == ZG:
                    build_h_wave(0)
                emit_block(0, 0)
                if li == ZG:
                    build_h_wave(1)
                insert = 0 < li < len(DIMS) - 1 and mt <= 2
                if insert:
                    # phase n1's first wave-0 chunks slot in where phase n0
                    # would otherwise stall on the wave-1 gather landing
                    emit_block(0, 1, 0, 16)
                emit_block(1, 0)
                emit_epilogue(0)
                if gather_z:
                    zbounce = dram.tile([do, PH], BF16, tag="zb1",
                                        name=f"zb{li}_1")
                emit_block(0, 1, 16 if insert else 0, None)
                emit_block(1, 1)
                emit_epilogue(1)

                if not last:
                    h_tiles_cur = h_next
                    h_read = make_reader(h_next, len(h_next) > 1)

    nc.compile()
    return nc


def kernel(**inputs):
    X = np.asarray(inputs["X"], dtype=np.float32)
    adj = np.asarray(inputs["adj_"], dtype=np.float32)

    if "nc" not in _CACHED:
        _CACHED["nc"] = _build()
    nc = _CACHED["nc"]

    xT_full = np.ascontiguousarray(X.T).astype(NP_BF16)
    ws = [np.asarray(inputs[f"W{j + 1}"], np.float32).astype(NP_BF16) for j in range(6)]
    in_maps = []
    for i in range(NCORES):
        rows = slice(i * R, (i + 1) * R)
        m = {
            "adjT": np.ascontiguousarray(adj[rows, :].T).astype(NP_BF16),
            "xT": xT_full,
        }
        for j in range(6):
            m[f"W{j + 1}"] = ws[j]
        in_maps.append(m)

    res = run_bass_kernel_spmd(nc, in_maps, core_ids=list(range(NCORES)))
    out = np.concatenate(
        [np.asarray(r["outT"], dtype=np.float32).T for r in res.results], axis=0
    )
    return out



# revision 7
# speedup vs baseline: 1.2159x; 1.2159x over previous
"""GCN autoencoder (6x gcn_layer) on 8 TRN2 NeuronCores.

Strategy (v2):
  - Rows of adj_/X sharded across 8 cores; weights replicated; bf16 on
    device (fp32 PSUM), host does sharding / transposes / casts.
  - Reassociation: layers whose W *expands* width are computed as
    relu((A @ z) @ W) instead of relu(A @ (z W)) so the big adj-matmul
    always contracts against the narrower operand:
        l1: A@(X W1)    256 cols   (H-form, H1 local from replicated X)
        l2: A@(z1 W2)   256        (H-form)
        l3: A@(z2 W3)   128        (H-form)
        l4: (A@z3) W4   128        (z-form: gather z3, W4 deferred)
        l5: A@(z4 W5)   256        (H-form)
        l6: (A@z5) W6   256        (z-form: gather z5, W6 deferred)
    1280 adj-matmul columns/row-block vs 1664 unassociated (-23%), and
    the l4/l6 gathers shrink to 128/256 cols.
  - z-form carriers are emitted row-major by an identity-matmul
    transpose in the producing layer's epilogue (zT chunk @ I128).
  - adjT k-chunks 0:48 SBUF-resident (loaded once on the ACT DMA ring);
    chunks 48:64 streamed per layer per phase on the SP ring.
  - Per-phase production waves + balanced insert (as baseline): each
    phase's epilogue AllGathers the next layer's carrier; the consumer
    accumulates k-chunks in arrival-wave order.
  - Gather-dependent SBUF loads ride the ACT HWDGE ring so a pending
    AllGather can never FIFO-block the adj-stream/bounce DMAs (SP ring).
"""

import sys

import numpy as np

if "/opt/trn_rl_repo" not in sys.path:
    sys.path.insert(0, "/opt/trn_rl_repo")

import ml_dtypes

import concourse.bacc as bacc
import concourse.tile as tile
from concourse import mybir
from concourse.bass_utils import run_bass_kernel_spmd

N = 8192
D_IN = 512
NCORES = 8
R = N // NCORES  # 1024 rows per core
DIMS = [(512, 256), (256, 256), (256, 128), (128, 256), (256, 256), (256, 512)]

BF16 = mybir.dt.bfloat16
F32 = mybir.dt.float32
NP_BF16 = ml_dtypes.bfloat16
RELU = mybir.ActivationFunctionType.Relu

KO = N // 128  # 64 k-chunks over the gather dim
RT = R // 128  # 8 local row tiles
NPH = 2
PH = R // NPH  # 512 rows per phase
HALF = RT // NPH  # 4 k-chunks each core contributes per phase

NRES = 48  # adjT k-chunks SBUF-resident; KO-NRES streamed per layer
INSERT = 16  # phase-1 wave-0 chunks slotted in before the wave-1 join

# per-layer adj-matmul carrier width (cols) and form
CW = [256, 256, 128, 128, 256, 256]
ZFORM = [False, False, False, True, False, True]  # deferred-W layers
# z width out of each layer (after deferred W where applicable)
ZW = [256, 256, 128, 256, 256, 512]

_CACHED = {}


def _build():
    nc = bacc.Bacc(
        "TRN2",
        target_bir_lowering=False,
        debug=False,
        enable_asserts=False,
        num_devices=NCORES,
    )

    adjT = nc.dram_tensor("adjT", [N, R], BF16, kind="ExternalInput")
    xT = nc.dram_tensor("xT", [D_IN, N], BF16, kind="ExternalInput")
    w_dram = [
        nc.dram_tensor(f"W{i + 1}", list(DIMS[i]), BF16, kind="ExternalInput")
        for i in range(6)
    ]
    i_dram = nc.dram_tensor("I128", [128, 128], BF16, kind="ExternalInput")
    outT = nc.dram_tensor("outT", [DIMS[-1][1], R], F32, kind="ExternalOutput")

    adjT_r = adjT.ap().rearrange("(ko p) r -> p ko r", p=128)
    xT_r = xT.ap().rearrange("(kx p) c -> p kx c", p=128)

    with tile.TileContext(nc) as tc:
        with (
            tc.tile_pool(name="adjres", bufs=1) as adjres_p,
            tc.tile_pool(name="adjstr", bufs=6) as adjstr_p,
            tc.tile_pool(name="wp", bufs=1) as wp,
            tc.tile_pool(name="xtp", bufs=2) as xtp,
            tc.tile_pool(name="cp", bufs=4) as cpool,
            tc.tile_pool(name="ztp", bufs=5) as ztp,
            tc.tile_pool(name="usb", bufs=2) as usbp,
            tc.tile_pool(name="hstage", bufs=4) as hstage,
            tc.tile_pool(name="ostage", bufs=3) as ostage,
            tc.tile_pool(name="psz", bufs=4, space="PSUM") as psz,
            tc.tile_pool(name="psu", bufs=2, space="PSUM") as psu,
            tc.tile_pool(name="psh", bufs=2, space="PSUM") as psh,
            tc.tile_pool(name="dram", bufs=1, space="DRAM") as dram,
        ):
            # ---- resident weights + identity ----
            w_sb = []
            for i, (di, do) in enumerate(DIMS):
                w_t = wp.tile([128, di // 128, do], BF16, name=f"w{i}_sb")
                nc.sync.dma_start(
                    w_t[:], w_dram[i].ap().rearrange("(kx p) n -> p kx n", p=128)
                )
                w_sb.append(w_t)
            i_sb = wp.tile([128, 128], BF16, name="i_sb")
            nc.sync.dma_start(i_sb[:], i_dram.ap())

            # warmup AllGathers: absorb the collective-stream first-use cost
            for wi, wrows in enumerate((16, PH)):
                wu_in = dram.tile([wrows, 256], BF16, tag=f"wu{wi}i",
                                  name=f"wu{wi}i")
                wu_out = dram.tile([NCORES * wrows, 256], BF16,
                                   addr_space="Shared", tag=f"wu{wi}o",
                                   name=f"wu{wi}o")
                nc.gpsimd.collective_compute(
                    "AllGather",
                    mybir.AluOpType.bypass,
                    ins=[wu_in[:].opt()],
                    outs=[wu_out[:].opt()],
                    replica_groups=[list(range(NCORES))],
                )

            # resident adjT chunks 0:NRES on the ACT ring (loaded once, in
            # consumption order); streamed chunks handled per layer below
            adj_res = adjres_p.tile([128, NRES, R], BF16, name="adj_res")
            for q in range(8):
                lo = q * (NRES // 8)
                hi = lo + NRES // 8
                nc.scalar.dma_start(adj_res[:, lo:hi, :], adjT_r[:, lo:hi, :])

            adj_stream_cache = {}

            def adj_mov(g, n):
                if g < NRES:
                    return adj_res[:, g, n * PH : (n + 1) * PH]
                grp = g // 2
                t = adj_stream_cache.get((grp, n))
                if t is None:
                    t = adjstr_p.tile([128, 2, PH], BF16, tag="adjs",
                                      name=f"as{grp}_{n}")
                    nc.sync.dma_start(
                        t[:], adjT_r[:, grp * 2 : grp * 2 + 2,
                                     n * PH : (n + 1) * PH]
                    )
                    adj_stream_cache[(grp, n)] = t
                return t[:, g % 2, :]

            # ---- C1 = H1 = X @ W1 computed fully on every core ----
            # two wave buffers [128, 32, 256]; chunk g -> C1[g//32][:, g%32]
            c_cur = [
                cpool.tile([128, KO // 2, 256], BF16, tag="c", name=f"c1_{w}")
                for w in range(NPH)
            ]
            for g0 in range(0, KO, 4):
                xt_t = xtp.tile([128, D_IN // 128, 512], BF16, tag="xt")
                nc.sync.dma_start(xt_t[:], xT_r[:, :, g0 * 128 : g0 * 128 + 512])
                for g in range(g0, g0 + 4):
                    ps_h = psh.tile([128, 256], F32, tag="psh")
                    for kx in range(D_IN // 128):
                        c = (g - g0) * 128
                        nc.tensor.matmul(
                            ps_h[:],
                            xt_t[:, kx, c : c + 128],
                            w_sb[0][:, kx, :],
                            start=(kx == 0),
                            stop=(kx == D_IN // 128 - 1),
                        )
                    nc.vector.tensor_copy(
                        c_cur[g // (KO // 2)][:, g % (KO // 2), :], ps_h[:]
                    )

            def c1_read(m, g):
                return c_cur[g // (KO // 2)][:, g % (KO // 2),
                                             m * 128 : (m + 1) * 128]

            c_read = c1_read

            # consumption waves: layer 1 in production order (g ascending);
            # layers >=2 by producer phase ({c*8 + n*4 + j, j<4} per phase n)
            waves_l1 = [list(range(KO // 2)), list(range(KO // 2, KO))]
            waves_g = [
                [c * RT + n * HALF + j
                 for c in range(NCORES) for j in range(HALF)]
                for n in range(NPH)
            ]

            for li in range(6):
                di, do = DIMS[li]
                last = li == 5
                mt = CW[li] // 128          # adj-mm output width /128
                mtz = ZW[li] // 128         # z width /128
                kwaves = waves_l1 if li == 0 else waves_g
                adj_stream_cache.clear()

                if not last:
                    # next layer's carrier buffers (written by epilogue AGs)
                    c_next = [
                        cpool.tile([128, KO // 2, 256], BF16, tag="c",
                                   name=f"c{li + 2}_{w}")
                        for w in range(NPH)
                    ]
                    cw_next = CW[li + 1]

                    def make_reader(c_tiles, cwn):
                        def rd(m, g):
                            c, r8 = g // RT, g % RT
                            w, j = r8 // HALF, r8 % HALF
                            return c_tiles[w][:, c * HALF + j,
                                              m * 128 : (m + 1) * 128]
                        return rd

                ps_zs = [[psz.tile([128, PH], F32, tag="psz",
                                   name=f"psz{li}_{n}_{m}")
                          for m in range(mt)] for n in range(NPH)]
                mm_cnt = [[0] * mt for _ in range(NPH)]

                def emit_block(wb, n, lo=0, hi=None):
                    for g in kwaves[wb][lo:hi]:
                        mov = adj_mov(g, n)
                        for m in range(mt):
                            nc.tensor.matmul(
                                ps_zs[n][m][:],
                                c_read(m, g),
                                mov,
                                start=(mm_cnt[n][m] == 0),
                                stop=(mm_cnt[n][m] == KO - 1),
                            )
                            mm_cnt[n][m] += 1

                def emit_epilogue(n):
                    # ---- produce this phase's zT tiles ----
                    if ZFORM[li]:
                        # u = A @ C (unrelu'd); z = relu(u @ W_deferred)
                        u_sb = usbp.tile([128, 2, PH], BF16, tag="usb")
                        for m in range(mt):
                            nc.vector.tensor_copy(
                                u_sb[:, m, :], ps_zs[n][m][:]
                            )
                        zt_p = []
                        for mo in range(mtz):
                            ps_c = psu.tile([128, PH], F32, tag="psu")
                            for kx in range(mt):
                                nc.tensor.matmul(
                                    ps_c[:],
                                    w_sb[li][:, kx, mo * 128 : (mo + 1) * 128],
                                    u_sb[:, kx, :],
                                    start=(kx == 0),
                                    stop=(kx == mt - 1),
                                )
                            if last:
                                o_st = ostage.tile([128, PH], F32, tag="ost")
                                nc.scalar.activation(o_st[:], ps_c[:], RELU)
                                nc.sync.dma_start(
                                    outT[mo * 128 : (mo + 1) * 128,
                                         n * PH : (n + 1) * PH],
                                    o_st[:],
                                )
                            else:
                                z_t = ztp.tile([128, PH], BF16, tag="zt",
                                               name=f"z{li + 1}_{mo}_{n}")
                                nc.scalar.activation(z_t[:], ps_c[:], RELU)
                                zt_p.append(z_t)
                        if last:
                            return
                    else:
                        zt_p = []
                        for m in range(mt):
                            z_t = ztp.tile([128, PH], BF16, tag="zt",
                                           name=f"z{li + 1}_{m}_{n}")
                            nc.scalar.activation(z_t[:], ps_zs[n][m][:], RELU)
                            zt_p.append(z_t)

                    # ---- build C_{l+2}'s source rows: bounce + AllGather ----
                    zform_next = ZFORM[li + 1]
                    do2 = ZW[li] if zform_next else DIMS[li + 1][1]
                    bounce = dram.tile([PH, do2], BF16, tag=f"hb{li}_{n}",
                                       name=f"hb{li}_{n}")
                    for j in range(HALF):
                        ps_h = psh.tile([128, 256], F32, tag="psh")
                        if zform_next:
                            # row-major z via identity transpose:
                            # ps_h[:, co*128:...] = (zT[co][:, j])^T
                            for co in range(mtz):
                                nc.tensor.matmul(
                                    ps_h[:, co * 128 : (co + 1) * 128],
                                    zt_p[co][:, j * 128 : (j + 1) * 128],
                                    i_sb[:],
                                    start=(co == 0),
                                    stop=(co == mtz - 1),
                                )
                        else:
                            for kx in range(mtz):
                                nc.tensor.matmul(
                                    ps_h[:, 0:do2],
                                    zt_p[kx][:, j * 128 : (j + 1) * 128],
                                    w_sb[li + 1][:, kx, :],
                                    start=(kx == 0),
                                    stop=(kx == mtz - 1),
                                )
                        h_st = hstage.tile([128, 256], BF16, tag="hst")
                        nc.vector.tensor_copy(h_st[:, 0:do2], ps_h[:, 0:do2])
                        nc.sync.dma_start(
                            bounce[j * 128 : (j + 1) * 128, :], h_st[:, 0:do2]
                        )
                    gath = dram.tile(
                        [NCORES * PH, do2], BF16, addr_space="Shared",
                        tag=f"hg{li}_{n}", name=f"hg{li}_{n}",
                    )
                    nc.gpsimd.collective_compute(
                        "AllGather",
                        mybir.AluOpType.bypass,
                        ins=[bounce[:].opt()],
                        outs=[gath[:].opt()],
                        replica_groups=[list(range(NCORES))],
                    )
                    # gather-dependent loads on the ACT ring (per core c so
                    # early chunks unblock as they land)
                    g_r = gath.rearrange("(c j p) d -> p (c j) d", p=128,
                                         c=NCORES)
                    for c in range(NCORES):
                        nc.scalar.dma_start(
                            c_next[n][:, c * HALF : (c + 1) * HALF, 0:do2],
                            g_r[:, c * HALF : (c + 1) * HALF, :],
                        )

                emit_block(0, 0)
                insert = 0 < li and mt <= 2
                if insert:
                    emit_block(0, 1, 0, INSERT)
                emit_block(1, 0)
                emit_epilogue(0)
                emit_block(0, 1, INSERT if insert else 0, None)
                emit_block(1, 1)
                emit_epilogue(1)

                if not last:
                    c_read = make_reader(c_next, cw_next)

    nc.compile()
    return nc


def make_in_maps(inputs):
    X = np.asarray(inputs["X"], dtype=np.float32)
    adj = np.asarray(inputs["adj_"], dtype=np.float32)
    xT_full = np.ascontiguousarray(X.T).astype(NP_BF16)
    ws = [np.asarray(inputs[f"W{j + 1}"], np.float32).astype(NP_BF16)
          for j in range(6)]
    eye = np.eye(128, dtype=NP_BF16)
    in_maps = []
    for i in range(NCORES):
        rows = slice(i * R, (i + 1) * R)
        m = {
            "adjT": np.ascontiguousarray(adj[rows, :].T).astype(NP_BF16),
            "xT": xT_full,
            "I128": eye,
        }
        for j in range(6):
            m[f"W{j + 1}"] = ws[j]
        in_maps.append(m)
    return in_maps


def kernel(**inputs):
    if "nc" not in _CACHED:
        _CACHED["nc"] = _build()
    nc = _CACHED["nc"]

    res = run_bass_kernel_spmd(nc, make_in_maps(inputs),
                               core_ids=list(range(NCORES)))
    out = np.concatenate(
        [np.asarray(r["outT"], dtype=np.float32).T for r in res.results], axis=0
    )
    return out
